# revision 58
# baseline (speedup 1.0000x reference)
"""Trainium2 Bass kernel for LoRALinear: out = x @ W.T + b + scale*(x @ A.T) @ B.T.

Strategy
--------
* Host folds the LoRA path into the base weights (free prep):
      W' = W + scale * B @ A         [out_f, in_f]
  so the device computes a single dense matmul; bias + final descale are
  applied on the host after the gather.
* 8-way data-parallel over the flattened (batch*seq) rows: 16384 rows ->
  2048 rows per NeuronCore.  W' is replicated; no collectives.
* Each core computes the transposed output block  outT = W' @ x_shard.T.
* The whole contraction runs as fp8-e4m3 "DoubleRow" matmuls (2
  contraction rows packed per partition, 2x PE throughput).  Each
  128-deep k-tile is packed twice with a complementary quantization pair
  (a = q(v), b = q(2v - a)) on BOTH operands, so the slot-sum equals
  twice the pair-midpoint and halves each operand's quantization error
  (optimal 2-point midpoint encoding).  Net: ~1.6e-2 rel err at half the
  bf16 cycle cost.
* Operands are pre-scaled (x*4, W'*64) so fp8-e4m3 sees a well-centered
  dynamic range; the pair-sum doubling makes the effective weight scale
  128, and the single 1/512 descale is applied on host.
* Schedule: x streams on the SWDGE queue (whole k-tiles; one DMA each,
  since SWDGE descriptor generation costs ~1us per transfer) while the
  first 4 output blocks run as quarter-K "generations" whose partial
  sums accumulate in SBUF (DVE adds) -- with only 8 PSUM banks, at most
  2 full blocks could otherwise accumulate while x streams in, idling
  the PE ~40% of that phase.  Normal-block W prefetch rides the same
  SWDGE queue so it enqueues behind x in the DMA FIFO; outputs are
  written back in bf16.  The last block runs row-chunk-major so the
  final PSUM drain overlaps compute.
"""

import numpy as np
import ml_dtypes

import concourse.bass as bass
import concourse.bacc as bacc_mod
import concourse.mybir as mybir
import concourse.tile as tile
from concourse.bass_utils import run_bass_kernel_spmd

N_CORES = 8
P = 128
RF = 512  # moving free dim per matmul

IN_F = 4096
OUT_F = 4096
RANK = 8
SCALE = 8.0 / 8.0  # alpha / rank
B_DIM = 4
S_DIM = 4096
ROWS_TOTAL = B_DIM * S_DIM
ROWS = ROWS_TOTAL // N_CORES

KF8 = 4096            # contraction dims done in fp8 dither-pair DoubleRow
T8 = KF8 // P         # fp8 k-tiles (each packs the 128 dims twice)
TB = (IN_F - KF8) // P  # bf16 k-tiles (0: pure fp8)
NSPLIT = 5            # head blocks computed as quarter-K generations
QG = 4                # generations per split block
GT = T8 // QG         # k-tiles per generation
SX = 4.0              # x pre-scale for fp8 range
SW = 128.0            # effective W scale after the pair-sum doubling

BF16 = mybir.dt.bfloat16
FP8 = mybir.dt.float8e4
F32 = mybir.dt.float32
NP_BF16 = ml_dtypes.bfloat16
NP_FP8 = ml_dtypes.float8_e4m3


def _build(rows, in_f, out_f):
    """Build the per-core Bass program (same program for all cores)."""
    nb = out_f // P  # output-feature blocks (psum partition dim)
    rb = rows // RF  # row chunks (moving free dim)

    nc = bacc_mod.Bacc()
    x8 = nc.declare_dram_parameter("x8", [P, T8, 2, rows], FP8, isOutput=False)
    w8 = nc.declare_dram_parameter("w8", [nb, P, T8, 2, P], FP8, isOutput=False)
    if TB:
        xb = nc.declare_dram_parameter("xb", [P, TB, rows], BF16, isOutput=False)
        wb = nc.declare_dram_parameter("wb", [nb, P, TB, P], BF16, isOutput=False)
    # bf16 output: halves the writeback DMA; the quantization it adds is
    # ~0.2% rms, negligible next to the fp8 path's 1.6e-2.
    outT = nc.declare_dram_parameter("outT", [out_f, rows], BF16, isOutput=True)

    with tile.TileContext(nc) as tc:
        with (
            tc.tile_pool(name="xpool", bufs=1) as xpool,
            tc.tile_pool(name="wpool", bufs=2) as wpool,
            tc.tile_pool(name="wqpool", bufs=5) as wqpool,
            tc.tile_pool(name="opool", bufs=4) as opool,
            tc.tile_pool(name="ppool", bufs=NSPLIT * 4) as ppool,
            tc.tile_pool(name="mpsum", bufs=8, space="PSUM") as mpsum,
        ):
            # x resident in SBUF, one whole-tile DMA per k-tile (SWDGE
            # descriptor generation is ~1us per DMA, so finer chunking
            # makes the stream descriptor-bound).  gpsimd (SWDGE) queue
            # keeps the long x stream off the sync queue used for W.
            x8_sb = xpool.tile([P, T8, 2, rows], FP8)
            # t0 rides the sync queue (fast HWDGE descriptor gen); the
            # first generation's weight tiles lead the gpsimd queue so
            # their requests reach the DMA FIFO before the x stream.
            nc.sync.dma_start(x8_sb[:, 0], x8[:, 0])
            wq0 = {}
            for n in (0, 1):
                wq0[n] = wqpool.tile([P, GT, 2, P], FP8, name="wq", tag="wq")
                nc.gpsimd.dma_start(wq0[n], w8[n, :, 0:GT])
            for t in range(1, T8):
                nc.gpsimd.dma_start(x8_sb[:, t], x8[:, t])
            if TB:
                xb_sb = xpool.tile([P, TB, rows], BF16)
                for t in range(TB):
                    nc.gpsimd.dma_start(xb_sb[:, t], xb[:, t])

            # Head blocks: while x is still streaming in, only ~2 full
            # blocks' worth of PSUM accumulations can be open (8 banks),
            # which leaves the PE idle ~40% of the stream phase.  Compute
            # the first NSPLIT blocks in quarter-K generations instead:
            # each generation's 8 PSUM groups close after GT k-tiles and
            # the partial sums accumulate in SBUF (DVE add), so every
            # arrived x tile is reused by 4 blocks and the PE stays busy.
            assert TB == 0 or NSPLIT == 0
            part = {}
            for q in range(QG):
                for pair in ((0, 1), (2, 3), (4,)):
                    if q == 0 and pair == (0, 1):
                        wq = wq0
                    else:
                        wq = {}
                        for n in pair:
                            wq[n] = wqpool.tile(
                                [P, GT, 2, P], FP8, name="wq", tag="wq"
                            )
                            if q == QG - 1:
                                # last round's weights ride gpsimd, behind
                                # the x stream: they are needed late, and
                                # this keeps them out of the early DMA FIFO
                                nc.gpsimd.dma_start(
                                    wq[n], w8[n, :, q * GT : (q + 1) * GT]
                                )
                            else:
                                nc.sync.dma_start(
                                    wq[n], w8[n, :, q * GT : (q + 1) * GT]
                                )

                    ps = {
                        n: [
                            mpsum.tile([P, RF], F32, name="ps", tag="ps")
                            for _ in range(rb)
                        ]
                        for n in pair
                    }
                    for tt in range(GT):
                        for n in pair:
                            for r in range(rb):
                                nc.tensor.matmul(
                                    ps[n][r],
                                    lhsT=wq[n][:, tt],
                                    rhs=x8_sb[
                                        :, q * GT + tt, :,
                                        r * RF : (r + 1) * RF,
                                    ],
                                    start=(tt == 0),
                                    stop=(tt == GT - 1),
                                    perf_mode=mybir.MatmulPerfMode.DoubleRow,
                                )
                    for n in pair:
                        for r in range(rb):
                            if q == 0:
                                pt = ppool.tile(
                                    [P, RF], F32, name="pt", tag="pt"
                                )
                                nc.vector.tensor_copy(out=pt, in_=ps[n][r])
                                part[n, r] = pt
                            elif q < QG - 1:
                                nc.vector.tensor_tensor(
                                    part[n, r], part[n, r], ps[n][r],
                                    mybir.AluOpType.add,
                                )
                            else:
                                o_sb = opool.tile(
                                    [P, RF], BF16, name="o_sb", tag="o_sb"
                                )
                                nc.vector.tensor_tensor(
                                    o_sb, part[n, r], ps[n][r],
                                    mybir.AluOpType.add,
                                )
                                nc.sync.dma_start(
                                    outT[
                                        n * P : (n + 1) * P,
                                        r * RF : (r + 1) * RF,
                                    ],
                                    o_sb,
                                )

            for n in range(NSPLIT, nb):
                w8_sb = wpool.tile([P, T8, 2, P], FP8, name="w8_sb", tag="w8_sb")
                # gpsimd queue: enqueues behind the x stream in the DMA
                # FIFO, so W prefetch cannot delay x during the split phase.
                # Halves let the block start once the lo half lands.
                h = T8 // 2
                nc.gpsimd.dma_start(w8_sb[:, :h], w8[n, :, :h])
                nc.gpsimd.dma_start(w8_sb[:, h:], w8[n, :, h:])
                if TB:
                    wb_sb = wpool.tile([P, TB, P], BF16, name="wb_sb", tag="wb_sb")
                    nc.sync.dma_start(wb_sb, wb[n])
                psums = (
                    [
                        mpsum.tile([P, RF], F32, name="ps", tag="ps")
                        for _ in range(rb)
                    ]
                    if n < nb - 1
                    else []
                )
                npool = mpsum

                def fp8_mm(t, r):
                    nc.tensor.matmul(
                        psums[r],
                        lhsT=w8_sb[:, t],
                        rhs=x8_sb[:, t, :, r * RF : (r + 1) * RF],
                        start=(t == 0),
                        stop=(TB == 0 and t == T8 - 1),
                        perf_mode=mybir.MatmulPerfMode.DoubleRow,
                    )

                def bf16_mm(t, r):
                    nc.tensor.matmul(
                        psums[r],
                        lhsT=wb_sb[:, t],
                        rhs=xb_sb[:, t, r * RF : (r + 1) * RF],
                        start=False,
                        stop=(t == TB - 1),
                    )

                if n < nb - 1:
                    # k-tile-major: consumes x tiles in DMA arrival order.
                    for t in range(T8):
                        for r in range(rb):
                            fp8_mm(t, r)
                    for t in range(TB):
                        for r in range(rb):
                            bf16_mm(t, r)
                    for r in range(rb):
                        o_sb = opool.tile([P, RF], BF16, name="o_sb", tag="o_sb")
                        nc.vector.tensor_copy(out=o_sb, in_=psums[r])
                        nc.sync.dma_start(
                            outT[n * P : (n + 1) * P, r * RF : (r + 1) * RF], o_sb
                        )
                else:
                    # last block: row-chunk-major so each PSUM drains while
                    # the next row chunk still computes; the final chunks
                    # are narrow so the post-last-matmul drain is short.
                    chunks = [(0, RF), (RF, RF), (2 * RF, RF),
                              (3 * RF, RF // 2), (3 * RF + RF // 2, RF // 2)]
                    for ci, (c0, cw) in enumerate(chunks):
                        pc = npool.tile([P, cw], F32, name="ps", tag="ps")
                        for t in range(T8):
                            nc.tensor.matmul(
                                pc,
                                lhsT=w8_sb[:, t],
                                rhs=x8_sb[:, t, :, c0 : c0 + cw],
                                start=(t == 0),
                                stop=(t == T8 - 1),
                                perf_mode=mybir.MatmulPerfMode.DoubleRow,
                            )
                        o_sb = opool.tile([P, cw], BF16, name="o_sb", tag="o_sb")
                        nc.vector.tensor_copy(out=o_sb, in_=pc)
                        nc.sync.dma_start(
                            outT[n * P : (n + 1) * P, c0 : c0 + cw], o_sb
                        )
    nc.finalize()
    return nc


def _pair_quant(v):
    """Complementary fp8 pair: midpoint of (a, b) is the best 2-point
    e4m3 approximation of v."""
    a = v.astype(NP_FP8)
    b = (2.0 * v - a.astype(np.float32)).astype(NP_FP8)
    return a, b


def _prep_shared(W, b, lora_A, lora_B, in_f, out_f):
    nb = out_f // P
    W64 = W.astype(np.float64) + SCALE * (
        lora_B.astype(np.float64) @ lora_A.astype(np.float64)
    )
    Wsc = (W64 * SW).astype(np.float32)  # [out_f, in_f], pre-scaled

    # fp8 part at HALF weight scale: the DoubleRow slot-sum Wa*xa + Wb*xb
    # is twice the pair midpoint, so quantizing W at SW/2 makes the fp8
    # contribution land on the same SX*SW scale as the bf16 part.
    # w8[n, p, t, s, m] = quant_s(0.5 * Wsc[n*128+m, t*128+p])
    Wf8 = 0.5 * Wsc[:, :KF8].T.reshape(T8, P, nb, P)  # [t, p, n, m]
    wa, wb_ = _pair_quant(Wf8)
    w8 = np.empty((nb, P, T8, 2, P), NP_FP8)
    w8[:, :, :, 0, :] = wa.transpose(2, 1, 0, 3)
    w8[:, :, :, 1, :] = wb_.transpose(2, 1, 0, 3)

    if not TB:
        return w8, None
    # bf16 part: wb[n, p, t, m] = Wsc[n*128+m, KF8 + t*128+p]
    wbf = (
        Wsc[:, KF8:].T.reshape(TB, P, nb, P).transpose(2, 1, 0, 3).astype(NP_BF16)
    )
    return w8, np.ascontiguousarray(wbf)


def _prep_x_shard(x2d, core, rows, in_f):
    xs = x2d[core * rows : (core + 1) * rows] * SX  # [rows, in_f] pre-scaled

    xf8 = xs[:, :KF8].T.reshape(T8, P, rows)  # [t, p, r]
    xa, xb_ = _pair_quant(xf8.astype(np.float32))
    x8 = np.empty((P, T8, 2, rows), NP_FP8)
    x8[:, :, 0, :] = xa.transpose(1, 0, 2)
    x8[:, :, 1, :] = xb_.transpose(1, 0, 2)

    if not TB:
        return x8, None
    xbf = (
        xs[:, KF8:].T.reshape(TB, P, rows).transpose(1, 0, 2).astype(NP_BF16)
    )
    return x8, np.ascontiguousarray(xbf)


def _prepare(x, W, b, lora_A, lora_B):
    """Build the Bass module and per-core input maps for these inputs."""
    x = np.asarray(x, np.float32)
    W = np.asarray(W, np.float32)
    b = np.asarray(b, np.float32)
    lora_A = np.asarray(lora_A, np.float32)
    lora_B = np.asarray(lora_B, np.float32)

    rows_total = x.shape[0] * x.shape[1] if x.ndim == 3 else x.shape[0]
    in_f = x.shape[-1]
    out_f = W.shape[0]
    rows = rows_total // N_CORES
    x2d = np.ascontiguousarray(x.reshape(rows_total, in_f))

    nc = _build(rows, in_f, out_f)
    w8, wbf = _prep_shared(W, b, lora_A, lora_B, in_f, out_f)
    in_maps = []
    for c in range(N_CORES):
        x8, xbf = _prep_x_shard(x2d, c, rows, in_f)
        m = {"x8": x8, "w8": w8}
        if TB:
            m["xb"] = xbf
            m["wb"] = wbf
        in_maps.append(m)
    return nc, in_maps, (rows_total, rows, out_f, x.shape, b)


def _run(x, W, b, lora_A, lora_B, trace=False, trace_kwargs=None):
    nc, in_maps, (rows_total, rows, out_f, xshape, bias) = _prepare(
        x, W, b, lora_A, lora_B
    )

    kwargs = {}
    if trace:
        kwargs["trace"] = True
        if trace_kwargs:
            kwargs["trace_kwargs"] = trace_kwargs
    res = run_bass_kernel_spmd(nc, in_maps, list(range(N_CORES)), **kwargs)

    inv = 1.0 / (SX * SW)
    out = np.empty((rows_total, out_f), np.float32)
    for c in range(N_CORES):
        out[c * rows : (c + 1) * rows] = (
            res.results[c]["outT"].astype(np.float32).T
        )
    out *= inv
    out += bias
    if len(xshape) == 3:
        out = out.reshape(xshape[0], xshape[1], out_f)
    return out, res


def kernel(x, W, b, lora_A, lora_B):
    out, _ = _run(x, W, b, lora_A, lora_B, trace=False)
    return out


# revision 64
# speedup vs baseline: 1.0030x; 1.0030x over previous
"""Trainium2 Bass kernel for LoRALinear: out = x @ W.T + b + scale*(x @ A.T) @ B.T.

Strategy
--------
* Host folds the LoRA path into the base weights (free prep):
      W' = W + scale * B @ A         [out_f, in_f]
  so the device computes a single dense matmul; bias + final descale are
  applied on the host after the gather.
* 8-way data-parallel over the flattened (batch*seq) rows: 16384 rows ->
  2048 rows per NeuronCore.  W' is replicated; no collectives.
* Each core computes the transposed output block  outT = W' @ x_shard.T.
* The whole contraction runs as fp8-e4m3 "DoubleRow" matmuls (2
  contraction rows packed per partition, 2x PE throughput).  Each
  128-deep k-tile is packed twice with a complementary quantization pair
  (a = q(v), b = q(2v - a)) on BOTH operands, so the slot-sum equals
  twice the pair-midpoint and halves each operand's quantization error
  (optimal 2-point midpoint encoding).  Net: ~1.6e-2 rel err at half the
  bf16 cycle cost.
* Operands are pre-scaled (x*4, W'*64) so fp8-e4m3 sees a well-centered
  dynamic range; the pair-sum doubling makes the effective weight scale
  128, and the single 1/512 descale is applied on host.
* Schedule: x streams on the SWDGE queue (whole k-tiles; one DMA each,
  since SWDGE descriptor generation costs ~1us per transfer) while the
  first 4 output blocks run as quarter-K "generations" whose partial
  sums accumulate in SBUF (DVE adds) -- with only 8 PSUM banks, at most
  2 full blocks could otherwise accumulate while x streams in, idling
  the PE ~40% of that phase.  Normal-block W prefetch rides the same
  SWDGE queue so it enqueues behind x in the DMA FIFO; outputs are
  written back in bf16.  The last block runs row-chunk-major so the
  final PSUM drain overlaps compute.
"""

import numpy as np
import ml_dtypes

import concourse.bass as bass
import concourse.bacc as bacc_mod
import concourse.mybir as mybir
import concourse.tile as tile
from concourse.bass_utils import run_bass_kernel_spmd

N_CORES = 8
P = 128
RF = 512  # moving free dim per matmul

IN_F = 4096
OUT_F = 4096
RANK = 8
SCALE = 8.0 / 8.0  # alpha / rank
B_DIM = 4
S_DIM = 4096
ROWS_TOTAL = B_DIM * S_DIM
ROWS = ROWS_TOTAL // N_CORES

KF8 = 4096            # contraction dims done in fp8 dither-pair DoubleRow
T8 = KF8 // P         # fp8 k-tiles (each packs the 128 dims twice)
TB = (IN_F - KF8) // P  # bf16 k-tiles (0: pure fp8)
NSPLIT = 5            # head blocks computed as quarter-K generations
QG = 4                # generations per split block
GT = T8 // QG         # k-tiles per generation
SX = 4.0              # x pre-scale for fp8 range
SW = 128.0            # effective W scale after the pair-sum doubling

BF16 = mybir.dt.bfloat16
FP8 = mybir.dt.float8e4
F32 = mybir.dt.float32
NP_BF16 = ml_dtypes.bfloat16
NP_FP8 = ml_dtypes.float8_e4m3


def _build(rows, in_f, out_f):
    """Build the per-core Bass program (same program for all cores)."""
    nb = out_f // P  # output-feature blocks (psum partition dim)
    rb = rows // RF  # row chunks (moving free dim)

    nc = bacc_mod.Bacc()
    x8 = nc.declare_dram_parameter("x8", [P, T8, 2, rows], FP8, isOutput=False)
    w8 = nc.declare_dram_parameter("w8", [nb, P, T8, 2, P], FP8, isOutput=False)
    if TB:
        xb = nc.declare_dram_parameter("xb", [P, TB, rows], BF16, isOutput=False)
        wb = nc.declare_dram_parameter("wb", [nb, P, TB, P], BF16, isOutput=False)
    # bf16 output: halves the writeback DMA; the quantization it adds is
    # ~0.2% rms, negligible next to the fp8 path's 1.6e-2.
    outT = nc.declare_dram_parameter("outT", [out_f, rows], BF16, isOutput=True)

    with tile.TileContext(nc) as tc:
        with (
            tc.tile_pool(name="xpool", bufs=1) as xpool,
            tc.tile_pool(name="wpool", bufs=2) as wpool,
            tc.tile_pool(name="wqpool", bufs=7) as wqpool,
            tc.tile_pool(name="opool", bufs=6) as opool,
            tc.tile_pool(name="ppool", bufs=NSPLIT * 4) as ppool,
            tc.tile_pool(name="mpsum", bufs=8, space="PSUM") as mpsum,
        ):
            # x resident in SBUF, one whole-tile DMA per k-tile (SWDGE
            # descriptor generation is ~1us per DMA, so finer chunking
            # makes the stream descriptor-bound).  gpsimd (SWDGE) queue
            # keeps the long x stream off the sync queue used for W.
            x8_sb = xpool.tile([P, T8, 2, rows], FP8)
            # t0 rides the sync queue (fast HWDGE descriptor gen); the
            # first generation's weight tiles lead the gpsimd queue so
            # their requests reach the DMA FIFO before the x stream.
            nc.sync.dma_start(x8_sb[:, 0], x8[:, 0])
            wq0 = {}
            for n in (0, 1):
                wq0[n] = wqpool.tile([P, GT, 2, P], FP8, name="wq", tag="wq")
                nc.gpsimd.dma_start(wq0[n], w8[n, :, 0:GT])
            for t in range(1, T8):
                nc.gpsimd.dma_start(x8_sb[:, t], x8[:, t])
            if TB:
                xb_sb = xpool.tile([P, TB, rows], BF16)
                for t in range(TB):
                    nc.gpsimd.dma_start(xb_sb[:, t], xb[:, t])

            # Head blocks: while x is still streaming in, only ~2 full
            # blocks' worth of PSUM accumulations can be open (8 banks),
            # which leaves the PE idle ~40% of the stream phase.  Compute
            # the first NSPLIT blocks in quarter-K generations instead:
            # each generation's 8 PSUM groups close after GT k-tiles and
            # the partial sums accumulate in SBUF (DVE add), so every
            # arrived x tile is reused by 4 blocks and the PE stays busy.
            assert TB == 0 or NSPLIT == 0
            part = {}
            for q in range(QG):
                for pair in ((0, 1), (2, 3), (4,)):
                    if q == 0 and pair == (0, 1):
                        wq = wq0
                    else:
                        wq = {}
                        for n in pair:
                            wq[n] = wqpool.tile(
                                [P, GT, 2, P], FP8, name="wq", tag="wq"
                            )
                            if q == QG - 1:
                                # last round's weights ride gpsimd, behind
                                # the x stream: they are needed late, and
                                # this keeps them out of the early DMA FIFO
                                nc.gpsimd.dma_start(
                                    wq[n], w8[n, :, q * GT : (q + 1) * GT]
                                )
                            else:
                                nc.sync.dma_start(
                                    wq[n], w8[n, :, q * GT : (q + 1) * GT]
                                )

                    ps = {
                        n: [
                            mpsum.tile([P, RF], F32, name="ps", tag="ps")
                            for _ in range(rb)
                        ]
                        for n in pair
                    }
                    for tt in range(GT):
                        for n in pair:
                            for r in range(rb):
                                nc.tensor.matmul(
                                    ps[n][r],
                                    lhsT=wq[n][:, tt],
                                    rhs=x8_sb[
                                        :, q * GT + tt, :,
                                        r * RF : (r + 1) * RF,
                                    ],
                                    start=(tt == 0),
                                    stop=(tt == GT - 1),
                                    perf_mode=mybir.MatmulPerfMode.DoubleRow,
                                )
                    for n in pair:
                        for r in range(rb):
                            if q == 0:
                                pt = ppool.tile(
                                    [P, RF], F32, name="pt", tag="pt"
                                )
                                nc.vector.tensor_copy(out=pt, in_=ps[n][r])
                                part[n, r] = pt
                            elif q < QG - 1:
                                nc.vector.tensor_tensor(
                                    part[n, r], part[n, r], ps[n][r],
                                    mybir.AluOpType.add,
                                )
                            else:
                                o_sb = opool.tile(
                                    [P, RF], BF16, name="o_sb", tag="o_sb"
                                )
                                nc.vector.tensor_tensor(
                                    o_sb, part[n, r], ps[n][r],
                                    mybir.AluOpType.add,
                                )
                                nc.sync.dma_start(
                                    outT[
                                        n * P : (n + 1) * P,
                                        r * RF : (r + 1) * RF,
                                    ],
                                    o_sb,
                                )

            for n in range(NSPLIT, nb):
                w8_sb = wpool.tile([P, T8, 2, P], FP8, name="w8_sb", tag="w8_sb")
                # gpsimd queue: enqueues behind the x stream in the DMA
                # FIFO, so W prefetch cannot delay x during the split phase.
                # Halves let the block start once the lo half lands.
                h = T8 // 2
                nc.gpsimd.dma_start(w8_sb[:, :h], w8[n, :, :h])
                nc.gpsimd.dma_start(w8_sb[:, h:], w8[n, :, h:])
                if TB:
                    wb_sb = wpool.tile([P, TB, P], BF16, name="wb_sb", tag="wb_sb")
                    nc.sync.dma_start(wb_sb, wb[n])
                psums = (
                    [
                        mpsum.tile([P, RF], F32, name="ps", tag="ps")
                        for _ in range(rb)
                    ]
                    if n < nb - 1
                    else []
                )
                npool = mpsum

                def fp8_mm(t, r):
                    nc.tensor.matmul(
                        psums[r],
                        lhsT=w8_sb[:, t],
                        rhs=x8_sb[:, t, :, r * RF : (r + 1) * RF],
                        start=(t == 0),
                        stop=(TB == 0 and t == T8 - 1),
                        perf_mode=mybir.MatmulPerfMode.DoubleRow,
                    )

                def bf16_mm(t, r):
                    nc.tensor.matmul(
                        psums[r],
                        lhsT=wb_sb[:, t],
                        rhs=xb_sb[:, t, r * RF : (r + 1) * RF],
                        start=False,
                        stop=(t == TB - 1),
                    )

                if n < nb - 1:
                    # k-tile-major: consumes x tiles in DMA arrival order.
                    for t in range(T8):
                        for r in range(rb):
                            fp8_mm(t, r)
                    for t in range(TB):
                        for r in range(rb):
                            bf16_mm(t, r)
                    for r in range(rb):
                        o_sb = opool.tile([P, RF], BF16, name="o_sb", tag="o_sb")
                        nc.vector.tensor_copy(out=o_sb, in_=psums[r])
                        nc.sync.dma_start(
                            outT[n * P : (n + 1) * P, r * RF : (r + 1) * RF], o_sb
                        )
                else:
                    # last block: row-chunk-major so each PSUM drains while
                    # the next row chunk still computes; the final chunks
                    # are narrow so the post-last-matmul drain is short.
                    chunks = [(0, RF), (RF, RF), (2 * RF, RF),
                              (3 * RF, RF // 2), (3 * RF + RF // 2, RF // 2)]
                    for ci, (c0, cw) in enumerate(chunks):
                        pc = npool.tile([P, cw], F32, name="ps", tag="ps")
                        for t in range(T8):
                            nc.tensor.matmul(
                                pc,
                                lhsT=w8_sb[:, t],
                                rhs=x8_sb[:, t, :, c0 : c0 + cw],
                                start=(t == 0),
                                stop=(t == T8 - 1),
                                perf_mode=mybir.MatmulPerfMode.DoubleRow,
                            )
                        o_sb = opool.tile([P, cw], BF16, name="o_sb", tag="o_sb")
                        nc.vector.tensor_copy(out=o_sb, in_=pc)
                        nc.sync.dma_start(
                            outT[n * P : (n + 1) * P, c0 : c0 + cw], o_sb
                        )
    nc.finalize()
    return nc


def _pair_quant(v):
    """Complementary fp8 pair: midpoint of (a, b) is the best 2-point
    e4m3 approximation of v."""
    a = v.astype(NP_FP8)
    b = (2.0 * v - a.astype(np.float32)).astype(NP_FP8)
    return a, b


def _prep_shared(W, b, lora_A, lora_B, in_f, out_f):
    nb = out_f // P
    W64 = W.astype(np.float64) + SCALE * (
        lora_B.astype(np.float64) @ lora_A.astype(np.float64)
    )
    Wsc = (W64 * SW).astype(np.float32)  # [out_f, in_f], pre-scaled

    # fp8 part at HALF weight scale: the DoubleRow slot-sum Wa*xa + Wb*xb
    # is twice the pair midpoint, so quantizing W at SW/2 makes the fp8
    # contribution land on the same SX*SW scale as the bf16 part.
    # w8[n, p, t, s, m] = quant_s(0.5 * Wsc[n*128+m, t*128+p])
    Wf8 = 0.5 * Wsc[:, :KF8].T.reshape(T8, P, nb, P)  # [t, p, n, m]
    wa, wb_ = _pair_quant(Wf8)
    w8 = np.empty((nb, P, T8, 2, P), NP_FP8)
    w8[:, :, :, 0, :] = wa.transpose(2, 1, 0, 3)
    w8[:, :, :, 1, :] = wb_.transpose(2, 1, 0, 3)

    if not TB:
        return w8, None
    # bf16 part: wb[n, p, t, m] = Wsc[n*128+m, KF8 + t*128+p]
    wbf = (
        Wsc[:, KF8:].T.reshape(TB, P, nb, P).transpose(2, 1, 0, 3).astype(NP_BF16)
    )
    return w8, np.ascontiguousarray(wbf)


def _prep_x_shard(x2d, core, rows, in_f):
    xs = x2d[core * rows : (core + 1) * rows] * SX  # [rows, in_f] pre-scaled

    xf8 = xs[:, :KF8].T.reshape(T8, P, rows)  # [t, p, r]
    xa, xb_ = _pair_quant(xf8.astype(np.float32))
    x8 = np.empty((P, T8, 2, rows), NP_FP8)
    x8[:, :, 0, :] = xa.transpose(1, 0, 2)
    x8[:, :, 1, :] = xb_.transpose(1, 0, 2)

    if not TB:
        return x8, None
    xbf = (
        xs[:, KF8:].T.reshape(TB, P, rows).transpose(1, 0, 2).astype(NP_BF16)
    )
    return x8, np.ascontiguousarray(xbf)


def _prepare(x, W, b, lora_A, lora_B):
    """Build the Bass module and per-core input maps for these inputs."""
    x = np.asarray(x, np.float32)
    W = np.asarray(W, np.float32)
    b = np.asarray(b, np.float32)
    lora_A = np.asarray(lora_A, np.float32)
    lora_B = np.asarray(lora_B, np.float32)

    rows_total = x.shape[0] * x.shape[1] if x.ndim == 3 else x.shape[0]
    in_f = x.shape[-1]
    out_f = W.shape[0]
    rows = rows_total // N_CORES
    x2d = np.ascontiguousarray(x.reshape(rows_total, in_f))

    nc = _build(rows, in_f, out_f)
    w8, wbf = _prep_shared(W, b, lora_A, lora_B, in_f, out_f)
    in_maps = []
    for c in range(N_CORES):
        x8, xbf = _prep_x_shard(x2d, c, rows, in_f)
        m = {"x8": x8, "w8": w8}
        if TB:
            m["xb"] = xbf
            m["wb"] = wbf
        in_maps.append(m)
    return nc, in_maps, (rows_total, rows, out_f, x.shape, b)


def _run(x, W, b, lora_A, lora_B, trace=False, trace_kwargs=None):
    nc, in_maps, (rows_total, rows, out_f, xshape, bias) = _prepare(
        x, W, b, lora_A, lora_B
    )

    kwargs = {}
    if trace:
        kwargs["trace"] = True
        if trace_kwargs:
            kwargs["trace_kwargs"] = trace_kwargs
    res = run_bass_kernel_spmd(nc, in_maps, list(range(N_CORES)), **kwargs)

    inv = 1.0 / (SX * SW)
    out = np.empty((rows_total, out_f), np.float32)
    for c in range(N_CORES):
        out[c * rows : (c + 1) * rows] = (
            res.results[c]["outT"].astype(np.float32).T
        )
    out *= inv
    out += bias
    if len(xshape) == 3:
        out = out.reshape(xshape[0], xshape[1], out_f)
    return out, res


def kernel(x, W, b, lora_A, lora_B):
    out, _ = _run(x, W, b, lora_A, lora_B, trace=False)
    return out


# revision 75
# speedup vs baseline: 1.0031x; 1.0001x over previous
"""Trainium2 Bass kernel for LoRALinear: out = x @ W.T + b + scale*(x @ A.T) @ B.T.

Strategy
--------
* Host folds the LoRA path into the base weights (free prep):
      W' = W + scale * B @ A         [out_f, in_f]
  so the device computes a single dense matmul; bias + final descale are
  applied on the host after the gather.
* 8-way data-parallel over the flattened (batch*seq) rows: 16384 rows ->
  2048 rows per NeuronCore.  W' is replicated; no collectives.
* Each core computes the transposed output block  outT = W' @ x_shard.T.
* The whole contraction runs as fp8-e4m3 "DoubleRow" matmuls (2
  contraction rows packed per partition, 2x PE throughput).  Each
  128-deep k-tile is packed twice with a complementary quantization pair
  (a = q(v), b = q(2v - a)) on BOTH operands, so the slot-sum equals
  twice the pair-midpoint and halves each operand's quantization error
  (optimal 2-point midpoint encoding).  Net: ~1.6e-2 rel err at half the
  bf16 cycle cost.
* Operands are pre-scaled (x*4, W'*64) so fp8-e4m3 sees a well-centered
  dynamic range; the pair-sum doubling makes the effective weight scale
  128, and the single 1/512 descale is applied on host.
* Schedule: x streams on the SWDGE queue (whole k-tiles; one DMA each,
  since SWDGE descriptor generation costs ~1us per transfer) while the
  first 4 output blocks run as quarter-K "generations" whose partial
  sums accumulate in SBUF (DVE adds) -- with only 8 PSUM banks, at most
  2 full blocks could otherwise accumulate while x streams in, idling
  the PE ~40% of that phase.  Normal-block W prefetch rides the same
  SWDGE queue so it enqueues behind x in the DMA FIFO; outputs are
  written back in bf16.  The last block runs row-chunk-major so the
  final PSUM drain overlaps compute.
"""

import numpy as np
import ml_dtypes

import concourse.bass as bass
import concourse.bacc as bacc_mod
import concourse.mybir as mybir
import concourse.tile as tile
from concourse.bass_utils import run_bass_kernel_spmd

N_CORES = 8
P = 128
RF = 512  # moving free dim per matmul

IN_F = 4096
OUT_F = 4096
RANK = 8
SCALE = 8.0 / 8.0  # alpha / rank
B_DIM = 4
S_DIM = 4096
ROWS_TOTAL = B_DIM * S_DIM
ROWS = ROWS_TOTAL // N_CORES

KF8 = 4096            # contraction dims done in fp8 dither-pair DoubleRow
T8 = KF8 // P         # fp8 k-tiles (each packs the 128 dims twice)
TB = (IN_F - KF8) // P  # bf16 k-tiles (0: pure fp8)
NSPLIT = 5            # head blocks computed as quarter-K generations
QG = 4                # generations per split block
GT = T8 // QG         # k-tiles per generation
SX = 4.0              # x pre-scale for fp8 range
SW = 128.0            # effective W scale after the pair-sum doubling

BF16 = mybir.dt.bfloat16
FP8 = mybir.dt.float8e4
F32 = mybir.dt.float32
NP_BF16 = ml_dtypes.bfloat16
NP_FP8 = ml_dtypes.float8_e4m3


def _build(rows, in_f, out_f):
    """Build the per-core Bass program (same program for all cores)."""
    nb = out_f // P  # output-feature blocks (psum partition dim)
    rb = rows // RF  # row chunks (moving free dim)

    nc = bacc_mod.Bacc()
    x8 = nc.declare_dram_parameter("x8", [P, T8, 2, rows], FP8, isOutput=False)
    w8 = nc.declare_dram_parameter("w8", [nb, P, T8, 2, P], FP8, isOutput=False)
    if TB:
        xb = nc.declare_dram_parameter("xb", [P, TB, rows], BF16, isOutput=False)
        wb = nc.declare_dram_parameter("wb", [nb, P, TB, P], BF16, isOutput=False)
    # bf16 output: halves the writeback DMA; the quantization it adds is
    # ~0.2% rms, negligible next to the fp8 path's 1.6e-2.
    outT = nc.declare_dram_parameter("outT", [out_f, rows], BF16, isOutput=True)

    with tile.TileContext(nc) as tc:
        with (
            tc.tile_pool(name="xpool", bufs=1) as xpool,
            tc.tile_pool(name="wpool", bufs=2) as wpool,
            tc.tile_pool(name="wqpool", bufs=7) as wqpool,
            tc.tile_pool(name="opool", bufs=6) as opool,
            tc.tile_pool(name="ppool", bufs=NSPLIT * 4) as ppool,
            tc.tile_pool(name="mpsum", bufs=8, space="PSUM") as mpsum,
        ):
            # x resident in SBUF, one whole-tile DMA per k-tile (SWDGE
            # descriptor generation is ~1us per DMA, so finer chunking
            # makes the stream descriptor-bound).  gpsimd (SWDGE) queue
            # keeps the long x stream off the sync queue used for W.
            x8_sb = xpool.tile([P, T8, 2, rows], FP8)
            # t0 rides the sync queue (fast HWDGE descriptor gen); the
            # first generation's weight tiles lead the gpsimd queue so
            # their requests reach the DMA FIFO before the x stream.
            nc.gpsimd.dma_start(x8_sb[:, 0], x8[:, 0])
            wq0 = {}
            for n in (0, 1):
                wq0[n] = wqpool.tile([P, GT, 2, P], FP8, name="wq", tag="wq")
                nc.sync.dma_start(wq0[n], w8[n, :, 0:GT])
            for t in range(1, T8):
                nc.gpsimd.dma_start(x8_sb[:, t], x8[:, t])
            if TB:
                xb_sb = xpool.tile([P, TB, rows], BF16)
                for t in range(TB):
                    nc.gpsimd.dma_start(xb_sb[:, t], xb[:, t])

            # Head blocks: while x is still streaming in, only ~2 full
            # blocks' worth of PSUM accumulations can be open (8 banks),
            # which leaves the PE idle ~40% of the stream phase.  Compute
            # the first NSPLIT blocks in quarter-K generations instead:
            # each generation's 8 PSUM groups close after GT k-tiles and
            # the partial sums accumulate in SBUF (DVE add), so every
            # arrived x tile is reused by 4 blocks and the PE stays busy.
            assert TB == 0 or NSPLIT == 0
            part = {}
            for q in range(QG):
                for pair in ((0, 1), (2, 3), (4,)):
                    if q == 0 and pair == (0, 1):
                        wq = wq0
                    else:
                        wq = {}
                        for n in pair:
                            wq[n] = wqpool.tile(
                                [P, GT, 2, P], FP8, name="wq", tag="wq"
                            )
                            if q == QG - 1:
                                # last round's weights ride gpsimd, behind
                                # the x stream: they are needed late, and
                                # this keeps them out of the early DMA FIFO
                                nc.gpsimd.dma_start(
                                    wq[n], w8[n, :, q * GT : (q + 1) * GT]
                                )
                            else:
                                nc.sync.dma_start(
                                    wq[n], w8[n, :, q * GT : (q + 1) * GT]
                                )

                    ps = {
                        n: [
                            mpsum.tile([P, RF], F32, name="ps", tag="ps")
                            for _ in range(rb)
                        ]
                        for n in pair
                    }
                    for tt in range(GT):
                        for n in pair:
                            for r in range(rb):
                                nc.tensor.matmul(
                                    ps[n][r],
                                    lhsT=wq[n][:, tt],
                                    rhs=x8_sb[
                                        :, q * GT + tt, :,
                                        r * RF : (r + 1) * RF,
                                    ],
                                    start=(tt == 0),
                                    stop=(tt == GT - 1),
                                    perf_mode=mybir.MatmulPerfMode.DoubleRow,
                                )
                    for n in pair:
                        for r in range(rb):
                            if q == 0:
                                pt = ppool.tile(
                                    [P, RF], F32, name="pt", tag="pt"
                                )
                                nc.vector.tensor_copy(out=pt, in_=ps[n][r])
                                part[n, r] = pt
                            elif q < QG - 1:
                                nc.vector.tensor_tensor(
                                    part[n, r], part[n, r], ps[n][r],
                                    mybir.AluOpType.add,
                                )
                            else:
                                o_sb = opool.tile(
                                    [P, RF], BF16, name="o_sb", tag="o_sb"
                                )
                                nc.vector.tensor_tensor(
                                    o_sb, part[n, r], ps[n][r],
                                    mybir.AluOpType.add,
                                )
                                nc.sync.dma_start(
                                    outT[
                                        n * P : (n + 1) * P,
                                        r * RF : (r + 1) * RF,
                                    ],
                                    o_sb,
                                )

            for n in range(NSPLIT, nb):
                w8_sb = wpool.tile([P, T8, 2, P], FP8, name="w8_sb", tag="w8_sb")
                # gpsimd queue: enqueues behind the x stream in the DMA
                # FIFO, so W prefetch cannot delay x during the split phase.
                # Halves let the block start once the lo half lands.
                h = T8 // 2
                nc.gpsimd.dma_start(w8_sb[:, :h], w8[n, :, :h])
                nc.gpsimd.dma_start(w8_sb[:, h:], w8[n, :, h:])
                if TB:
                    wb_sb = wpool.tile([P, TB, P], BF16, name="wb_sb", tag="wb_sb")
                    nc.sync.dma_start(wb_sb, wb[n])
                psums = (
                    [
                        mpsum.tile([P, RF], F32, name="ps", tag="ps")
                        for _ in range(rb)
                    ]
                    if n < nb - 1
                    else []
                )
                npool = mpsum

                def fp8_mm(t, r):
                    nc.tensor.matmul(
                        psums[r],
                        lhsT=w8_sb[:, t],
                        rhs=x8_sb[:, t, :, r * RF : (r + 1) * RF],
                        start=(t == 0),
                        stop=(TB == 0 and t == T8 - 1),
                        perf_mode=mybir.MatmulPerfMode.DoubleRow,
                    )

                def bf16_mm(t, r):
                    nc.tensor.matmul(
                        psums[r],
                        lhsT=wb_sb[:, t],
                        rhs=xb_sb[:, t, r * RF : (r + 1) * RF],
                        start=False,
                        stop=(t == TB - 1),
                    )

                if n < nb - 1:
                    # k-tile-major: consumes x tiles in DMA arrival order.
                    for t in range(T8):
                        for r in range(rb):
                            fp8_mm(t, r)
                    for t in range(TB):
                        for r in range(rb):
                            bf16_mm(t, r)
                    for r in range(rb):
                        o_sb = opool.tile([P, RF], BF16, name="o_sb", tag="o_sb")
                        nc.vector.tensor_copy(out=o_sb, in_=psums[r])
                        nc.sync.dma_start(
                            outT[n * P : (n + 1) * P, r * RF : (r + 1) * RF], o_sb
                        )
                else:
                    # last block: row-chunk-major so each PSUM drains while
                    # the next row chunk still computes; the final chunks
                    # are narrow so the post-last-matmul drain is short.
                    chunks = [(0, RF), (RF, RF), (2 * RF, RF),
                              (3 * RF, RF // 2), (3 * RF + RF // 2, RF // 2)]
                    for ci, (c0, cw) in enumerate(chunks):
                        pc = npool.tile([P, cw], F32, name="ps", tag="ps")
                        for t in range(T8):
                            nc.tensor.matmul(
                                pc,
                                lhsT=w8_sb[:, t],
                                rhs=x8_sb[:, t, :, c0 : c0 + cw],
                                start=(t == 0),
                                stop=(t == T8 - 1),
                                perf_mode=mybir.MatmulPerfMode.DoubleRow,
                            )
                        o_sb = opool.tile([P, cw], BF16, name="o_sb", tag="o_sb")
                        nc.vector.tensor_copy(out=o_sb, in_=pc)
                        nc.sync.dma_start(
                            outT[n * P : (n + 1) * P, c0 : c0 + cw], o_sb
                        )
    nc.finalize()
    return nc


def _pair_quant(v):
    """Complementary fp8 pair: midpoint of (a, b) is the best 2-point
    e4m3 approximation of v."""
    a = v.astype(NP_FP8)
    b = (2.0 * v - a.astype(np.float32)).astype(NP_FP8)
    return a, b


def _prep_shared(W, b, lora_A, lora_B, in_f, out_f):
    nb = out_f // P
    W64 = W.astype(np.float64) + SCALE * (
        lora_B.astype(np.float64) @ lora_A.astype(np.float64)
    )
    Wsc = (W64 * SW).astype(np.float32)  # [out_f, in_f], pre-scaled

    # fp8 part at HALF weight scale: the DoubleRow slot-sum Wa*xa + Wb*xb
    # is twice the pair midpoint, so quantizing W at SW/2 makes the fp8
    # contribution land on the same SX*SW scale as the bf16 part.
    # w8[n, p, t, s, m] = quant_s(0.5 * Wsc[n*128+m, t*128+p])
    Wf8 = 0.5 * Wsc[:, :KF8].T.reshape(T8, P, nb, P)  # [t, p, n, m]
    wa, wb_ = _pair_quant(Wf8)
    w8 = np.empty((nb, P, T8, 2, P), NP_FP8)
    w8[:, :, :, 0, :] = wa.transpose(2, 1, 0, 3)
    w8[:, :, :, 1, :] = wb_.transpose(2, 1, 0, 3)

    if not TB:
        return w8, None
    # bf16 part: wb[n, p, t, m] = Wsc[n*128+m, KF8 + t*128+p]
    wbf = (
        Wsc[:, KF8:].T.reshape(TB, P, nb, P).transpose(2, 1, 0, 3).astype(NP_BF16)
    )
    return w8, np.ascontiguousarray(wbf)


def _prep_x_shard(x2d, core, rows, in_f):
    xs = x2d[core * rows : (core + 1) * rows] * SX  # [rows, in_f] pre-scaled

    xf8 = xs[:, :KF8].T.reshape(T8, P, rows)  # [t, p, r]
    xa, xb_ = _pair_quant(xf8.astype(np.float32))
    x8 = np.empty((P, T8, 2, rows), NP_FP8)
    x8[:, :, 0, :] = xa.transpose(1, 0, 2)
    x8[:, :, 1, :] = xb_.transpose(1, 0, 2)

    if not TB:
        return x8, None
    xbf = (
        xs[:, KF8:].T.reshape(TB, P, rows).transpose(1, 0, 2).astype(NP_BF16)
    )
    return x8, np.ascontiguousarray(xbf)


def _prepare(x, W, b, lora_A, lora_B):
    """Build the Bass module and per-core input maps for these inputs."""
    x = np.asarray(x, np.float32)
    W = np.asarray(W, np.float32)
    b = np.asarray(b, np.float32)
    lora_A = np.asarray(lora_A, np.float32)
    lora_B = np.asarray(lora_B, np.float32)

    rows_total = x.shape[0] * x.shape[1] if x.ndim == 3 else x.shape[0]
    in_f = x.shape[-1]
    out_f = W.shape[0]
    rows = rows_total // N_CORES
    x2d = np.ascontiguousarray(x.reshape(rows_total, in_f))

    nc = _build(rows, in_f, out_f)
    w8, wbf = _prep_shared(W, b, lora_A, lora_B, in_f, out_f)
    in_maps = []
    for c in range(N_CORES):
        x8, xbf = _prep_x_shard(x2d, c, rows, in_f)
        m = {"x8": x8, "w8": w8}
        if TB:
            m["xb"] = xbf
            m["wb"] = wbf
        in_maps.append(m)
    return nc, in_maps, (rows_total, rows, out_f, x.shape, b)


def _run(x, W, b, lora_A, lora_B, trace=False, trace_kwargs=None):
    nc, in_maps, (rows_total, rows, out_f, xshape, bias) = _prepare(
        x, W, b, lora_A, lora_B
    )

    kwargs = {}
    if trace:
        kwargs["trace"] = True
        if trace_kwargs:
            kwargs["trace_kwargs"] = trace_kwargs
    res = run_bass_kernel_spmd(nc, in_maps, list(range(N_CORES)), **kwargs)

    inv = 1.0 / (SX * SW)
    out = np.empty((rows_total, out_f), np.float32)
    for c in range(N_CORES):
        out[c * rows : (c + 1) * rows] = (
            res.results[c]["outT"].astype(np.float32).T
        )
    out *= inv
    out += bias
    if len(xshape) == 3:
        out = out.reshape(xshape[0], xshape[1], out_f)
    return out, res


def kernel(x, W, b, lora_A, lora_B):
    out, _ = _run(x, W, b, lora_A, lora_B, trace=False)
    return out
